# revision 37
# baseline (speedup 1.0000x reference)
"""Multi-head attention (b=4, n=2048, dim=1024, heads=16) on 8 TRN2 cores.

Sharding: tensor-parallel over heads (2 heads per core) + row-parallel output
projection; host sums the 8 partial outputs and adds the bias.

Per-core pipeline (heads h0=2c, h1=2c+1):
  phase 1: qkv^T = w_in_c^T @ x^T (bf16), V transposed per 128-key tile
  phase 2: software-pipelined attention, chunk = (batch, 1024-query half):
      S^T   = k_h^T.T @ q_h^T          (PE, K=64)
      E^T   = exp(S^T / 8)             (512-wide halves; ACT exp for 21/32
                                        tiles, DVE Schraudolph bit-trick exp
                                        for 11/32, so neither engine outpaces
                                        the PE)
      [o^T; denom] = [v_h | 1].T @ E^T (PE, M=65, lags S by 6 j-slots so the
                                        PE never waits on exp -> HAM stays warm)
      o_norm^T = o^T * (1/denom)       (approx-reciprocal + DMA row-broadcast)
  phase 3: partial^T = w_out_c.T @ o_norm^T -> DRAM in bf16
"""

import os
import sys
import types

import numpy as np

# NTFF-profile hook shim: container's antenv lacks axon_hooks; harmless if
# tracing is never requested.
if "antenv.axon_hooks" not in sys.modules:
    try:
        from trn_agent_boot.trn_boot import _ntff_profile_via_ctypes

        _m = types.ModuleType("antenv.axon_hooks")
        _h = _ntff_profile_via_ctypes("/opt/axon/libaxon_pjrt.so")
        _m.get_axon_ntff_profile_hook = lambda: _h
        _m.set_axon_ntff_profile_hook = lambda hook: None
        sys.modules["antenv.axon_hooks"] = _m
    except Exception:
        pass

import ml_dtypes

import concourse.bacc as bacc
import concourse.bass as bass
import concourse.mybir as mybir
import concourse.tile as tile
from concourse.bass_utils import run_bass_kernel_spmd
from concourse.masks import make_identity

F32 = mybir.dt.float32
BF16 = mybir.dt.bfloat16
I16 = mybir.dt.int16

B, N, DIM, HEADS = 4, 2048, 1024, 16
HD = DIM // HEADS          # 64
NCORES = 8
HPC = HEADS // NCORES      # 2 heads per core
NT = B * N                 # 8192 tokens
MQKV = 3 * HPC * HD        # 384 qkv output dims per core
SCALE = HD ** -0.5         # 0.125

KT_TILES = DIM // 128      # 8 k-tiles in the projection contraction
NB1 = NT // 1024           # 8 n-blocks in phase 1
JT = N // 128              # 16 j-tiles per batch
NCHUNK = B * 2             # 8 (batch, ihalf) chunks
LAG = 6                    # PV lags S by this many (c, jt) slots

# Schraudolph exp-in-bf16: bits(int16) = round(x*SCALE*log2(e)*128 + B).
# C7=7.42 minimizes rms rel err (~1.6%); bias cancels in softmax normalize.
SCH_A = 128.0 * 1.4426950408889634 * SCALE
SCH_B = 127.0 * 128.0 - 7.42
# (jt, h) tiles per chunk handled by DVE instead of ACT (10 of 32)
if os.environ.get("KERNEL_NO_DVE_EXP"):
    DVE_EXP = set()
else:
    DVE_EXP = {(jt, 1) for jt in range(0, 16, 2)} | {(3, 1), (7, 1), (13, 1), (5, 0), (11, 0)}


def _build_nc():
    nc = bacc.Bacc("TRN2", target_bir_lowering=False, debug=False)

    xT = nc.dram_tensor("xT", [DIM, NT], BF16, kind="ExternalInput")
    w_in_c = nc.dram_tensor("w_in_c", [DIM, MQKV], BF16, kind="ExternalInput")
    w_out_c = nc.dram_tensor("w_out_c", [128, DIM], BF16, kind="ExternalInput")
    po = nc.dram_tensor("po", [DIM, NT], BF16, kind="ExternalOutput")
    rc_dram = nc.dram_tensor("rc_dram", [16, 1024], F32)

    with tile.TileContext(nc) as tc:
        with (
            tc.tile_pool(name="big", bufs=1) as big,
            tc.tile_pool(name="strm", bufs=2) as strm,
            tc.tile_pool(name="et", bufs=14) as etp,
            tc.tile_pool(name="ps", bufs=2, space="PSUM") as ps,
        ):
            # ---- persistent SBUF ----
            QT = big.tile([128, NT], BF16)    # [q_h0(0:64); q_h1(64:128)]^T
            # per-head K, zero-padded to 128 partitions so S matmuls use
            # the full-array PE config (no tile-mode switch vs PV)
            KT0 = big.tile([128, NT], BF16)
            KT1 = big.tile([128, NT], BF16)
            Vt = big.tile([128, B * JT, 130], BF16)  # [v_h0|1|v_h1|1] per j-tile
            o_sb = big.tile([128, NT], BF16)  # o^T both heads (normed in place)
            w_in_sb = big.tile([128, KT_TILES, MQKV], BF16)
            w_out_sb = big.tile([128, DIM], BF16)

            # interleave w_in/xin(0) issues per k so the k=0 slices (the
            # first matmul's operands) land before later descriptors queue
            xT_r = xT.rearrange("(kt p) n -> p kt n", p=128)
            w_in_r = w_in_c.rearrange("(kt p) m -> p kt m", p=128)
            xin0 = strm.tile([128, KT_TILES, 2, 512], BF16, tag="xin")
            for k in range(KT_TILES):
                nc.sync.dma_start(out=w_in_sb[:, k, :], in_=w_in_r[:, k, :])
                if k == 0:
                    for a in range(2):
                        nc.sync.dma_start(
                            out=xin0[:, 0, a, :],
                            in_=xT_r[:, 0, a * 512:(a + 1) * 512],
                        )
                else:
                    nc.sync.dma_start(
                        out=xin0[:, k, :, :],
                        in_=xT_r[:, k, 0:1024].rearrange(
                            "p (a b) -> p a b", b=512
                        ),
                    )
            nc.sync.dma_start(out=w_out_sb, in_=w_out_c[:, :])
            ident = big.tile([128, 128], BF16)
            make_identity(nc, ident)
            nc.vector.memset(Vt[:, :, 64], 1.0)
            nc.vector.memset(Vt[:, :, 129], 1.0)
            nc.gpsimd.memset(KT0[64:128, :], 0.0)
            nc.gpsimd.memset(KT1[0:64, :], 0.0)


            # ================= Phase 1: QKV projection =================
            # Per nb: 48 matmuls; transposes of nb-1 interleaved between the
            # m-groups so the PE never waits on the vstage copy.
            vstages = {}

            def emit_transposes(nbp, lo, hi):
                vst = vstages[nbp]
                for c in range(lo, hi):
                    g = nbp * 8 + c
                    tp = ps.tile(
                        [128, 128], BF16, tag="ps_o", bufs=2, name=f"tp{g}"
                    )
                    nc.tensor.transpose(
                        tp, vst[:, c * 128:(c + 1) * 128], ident
                    )
                    nc.scalar.copy(Vt[:, g, 0:64], tp[:, 0:64])
                    nc.scalar.copy(Vt[:, g, 65:129], tp[:, 64:128])

            for nb in range(NB1):
                ncol = slice(nb * 1024, (nb + 1) * 1024)
                if nb == 0:
                    xin = xin0
                else:
                    xin = strm.tile(
                        [128, KT_TILES, 2, 512], BF16, tag="xin"
                    )
                    for k in range(KT_TILES):
                        nc.sync.dma_start(
                            out=xin[:, k, :, :],
                            in_=xT_r[:, k, ncol].rearrange(
                                "p (a b) -> p a b", b=512
                            ),
                        )
                for m in range(3):
                    pjs = [
                        ps.tile(
                            [128, 512], F32, tag="ps_s", bufs=4,
                            name=f"pj{nb}_{m}_{a}",
                        )
                        for a in range(2)
                    ]
                    for k in range(KT_TILES):
                        for a in range(2):
                            nc.tensor.matmul(
                                pjs[a],
                                w_in_sb[:, k, m * 128:(m + 1) * 128],
                                xin[:, k, a, :],
                                start=(k == 0),
                                stop=(k == KT_TILES - 1),
                            )
                    if nb > 0:
                        emit_transposes(nb - 1, m * 3, min(m * 3 + 3, 8))
                    if m == 2:
                        vstage = strm.tile([128, 1024], BF16, tag="vstage")
                        vstages[nb] = vstage
                    for a in range(2):
                        acol = slice(
                            nb * 1024 + a * 512, nb * 1024 + (a + 1) * 512
                        )
                        if m == 0:
                            nc.vector.tensor_copy(QT[:, acol], pjs[a])
                        elif m == 1:
                            nc.vector.tensor_copy(
                                KT0[0:64, acol], pjs[a][0:64, :]
                            )
                            nc.vector.tensor_copy(
                                KT1[64:128, acol], pjs[a][64:128, :]
                            )
                        else:
                            nc.vector.tensor_copy(
                                vstage[:, a * 512:(a + 1) * 512], pjs[a]
                            )
            emit_transposes(NB1 - 1, 0, 8)

            # ====== Phase 2: software-pipelined attention ======
            # slot g = c*16 + jt; S+exp at slot g, PV at slot g-LAG.
            # chunk c = b*2 + ihalf; denom rows in dn_dram at c*2 + h.
            ets = {}
            po_h = {}
            NSLOT = NCHUNK * JT

            def emit_S(c, jt):
                b, ihalf = c // 2, c % 2
                i0 = b * N + ihalf * 1024
                jcol = slice(b * N + jt * 128, b * N + jt * 128 + 128)
                for h in range(HPC):
                    KTh = KT0 if h == 0 else KT1
                    et = etp.tile([128, 1024], BF16, tag="et", name="et")
                    for a in range(2):
                        st = ps.tile(
                            [128, 512], F32, tag="ps_s", bufs=4,
                            name=f"st{c}_{jt}_{h}_{a}",
                        )
                        nc.tensor.matmul(
                            st, KTh[:, jcol],
                            QT[:, i0 + a * 512:i0 + (a + 1) * 512],
                            start=True, stop=True,
                        )
                        eta = et[:, a * 512:(a + 1) * 512]
                        if (jt, h) in DVE_EXP:
                            nc.vector.tensor_scalar(
                                eta.bitcast(I16), st, SCH_A, SCH_B,
                                mybir.AluOpType.mult, mybir.AluOpType.add,
                            )
                        else:
                            nc.scalar.activation(
                                eta, st,
                                mybir.ActivationFunctionType.Exp,
                                scale=SCALE,
                            )
                    ets[(c, jt, h)] = et

            def emit_PV(c, jt):
                b = c // 2
                if jt == 0:
                    po_h[c] = [
                        ps.tile(
                            [65, 2, 512], F32, tag="ps_o", bufs=2,
                            name=f"po{c}_{h}",
                        )
                        for h in range(HPC)
                    ]
                for h in range(HPC):
                    et = ets.pop((c, jt, h))
                    for a in range(2):
                        nc.tensor.matmul(
                            po_h[c][h][:, a, :],
                            Vt[:, b * JT + jt, h * 65:h * 65 + 65],
                            et[:, a * 512:(a + 1) * 512],
                            start=(jt == 0), stop=(jt == JT - 1),
                        )
                if jt == JT - 1:
                    emit_drain(c)

            deferred = []

            def emit_drain(c):
                # Free po_h[c] banks fast: one full-tile copy each (ACT takes
                # h0, DVE takes h1) into f32 staging; the denom/normalize
                # chain is deferred into following slots' engine slack.
                b, ihalf = c // 2, c % 2
                icol = slice(b * N + ihalf * 1024, b * N + ihalf * 1024 + 1024)
                g0 = c * 2
                stg = [
                    strm.tile([65, 1024], F32, tag="ostg", bufs=3,
                              name=f"ostg{c}_{h}")
                    for h in range(HPC)
                ]
                nc.scalar.copy(
                    stg[0], po_h[c][0].rearrange("p a b -> p (a b)")
                )
                nc.vector.tensor_copy(
                    stg[1], po_h[c][1].rearrange("p a b -> p (a b)")
                )
                bcasts = [
                    strm.tile([64, 1024], F32, tag="bcast", bufs=2,
                              name=f"bc{c}_{h}")
                    for h in range(HPC)
                ]
                h1s = strm.tile([64, 1024], BF16, tag="h1s")

                def d_rc(h):
                    dn = strm.tile([1, 1024], F32, tag="dns", bufs=2,
                                   name=f"dn{c}_{h}")
                    nc.sync.dma_start(out=dn, in_=stg[h][64:65, :])
                    rc = strm.tile([1, 1024], F32, tag="rcs", bufs=2,
                                   name=f"rc{c}_{h}")
                    nc.vector.reciprocal_approx_fast(out=rc, in_=dn)
                    nc.sync.dma_start(
                        out=rc_dram[g0 + h:g0 + h + 1, :], in_=rc
                    )
                    src = rc_dram[g0 + h:g0 + h + 1, :]
                    rbc = bass.AP(
                        tensor=src.tensor,
                        offset=src.offset,
                        ap=[[0, 64]] + list(src.ap)[1:],
                    )
                    nc.sync.dma_start(out=bcasts[h], in_=rbc)

                def d_norm0():
                    nc.vector.tensor_mul(
                        o_sb[0:64, icol], stg[0][0:64, :], bcasts[0]
                    )

                def d_norm1():
                    nc.vector.tensor_mul(h1s, stg[1][0:64, :], bcasts[1])
                    nc.sync.dma_start(out=o_sb[64:128, icol], in_=h1s)

                deferred.extend(
                    [lambda: d_rc(0), lambda: d_rc(1), d_norm0, d_norm1]
                )

            def emit_proj(c, parity=None):
                b, ihalf = c // 2, c % 2
                i0p = b * N + ihalf * 1024
                for mt in range(DIM // 128):
                    if parity is not None and mt % 2 != parity:
                        continue
                    pout = strm.tile([128, 1024], BF16, tag="pout", bufs=6)
                    if mt % 2 == 0:
                        for a in range(2):
                            pp = ps.tile(
                                [128, 512], F32, tag="ps_s", bufs=4,
                                name=f"pp{c}_{mt}_{a}",
                            )
                            nc.tensor.matmul(
                                pp,
                                w_out_sb[:, mt * 128:(mt + 1) * 128],
                                o_sb[:, i0p + a * 512:i0p + (a + 1) * 512],
                                start=True, stop=True,
                            )
                            if a == 1 and mt % 4 == 0:
                                nc.scalar.copy(
                                    pout[:, a * 512:(a + 1) * 512], pp
                                )
                            else:
                                nc.vector.tensor_copy(
                                    pout[:, a * 512:(a + 1) * 512], pp
                                )
                    else:
                        pp = ps.tile(
                            [128, 2, 512], F32, tag="ps_o", bufs=2,
                            name=f"pp{c}_{mt}",
                        )
                        for a in range(2):
                            nc.tensor.matmul(
                                pp[:, a, :],
                                w_out_sb[:, mt * 128:(mt + 1) * 128],
                                o_sb[:, i0p + a * 512:i0p + (a + 1) * 512],
                                start=True, stop=True,
                            )
                        if mt == 3 and c % 2 == 0:
                            nc.vector.tensor_copy(
                                pout, pp.rearrange("p a b -> p (a b)")
                            )
                        else:
                            nc.scalar.copy(
                                pout, pp.rearrange("p a b -> p (a b)")
                            )
                    nc.sync.dma_start(
                        out=po[mt * 128:(mt + 1) * 128, i0p:i0p + 1024],
                        in_=pout,
                    )

            for g in range(NSLOT + LAG):
                if g < NSLOT:
                    emit_S(g // JT, g % JT)
                gp = g - LAG
                if gp >= 0:
                    emit_PV(gp // JT, gp % JT)
                if deferred:
                    deferred.pop(0)()
                if g == NSLOT:
                    emit_proj(0, parity=0)
                if g == NSLOT + 2:
                    emit_proj(1, parity=0)
                if g == NSLOT + 4:
                    emit_proj(2, parity=0)
                if g == NSLOT + 5:
                    emit_proj(3, parity=0)
            while deferred:
                deferred.pop(0)()

            # ====== Phase 3: output projection (dense, PE-warm) ======
            emit_proj(0, parity=1)
            emit_proj(1, parity=1)
            emit_proj(2, parity=1)
            emit_proj(3, parity=1)
            for c in range(4, NCHUNK):
                emit_proj(c)

    nc.finalize()
    return nc


_CACHED = {}


def kernel(x, w_in, w_out, b_out, _trace=False):
    if "nc" not in _CACHED:
        _CACHED["nc"] = _build_nc()
    nc = _CACHED["nc"]

    x2 = np.ascontiguousarray(
        x.reshape(NT, DIM).T.astype(np.float32)
    )  # [DIM, NT]
    in_maps = []
    for c in range(NCORES):
        h0, h1 = HPC * c, HPC * c + 1
        cols = []
        for part in range(3):  # q, k, v
            base = part * DIM
            cols.extend(range(base + h0 * HD, base + h0 * HD + HD))
            cols.extend(range(base + h1 * HD, base + h1 * HD + HD))
        w_in_cc = np.ascontiguousarray(w_in[:, cols].astype(np.float32))
        w_out_cc = np.ascontiguousarray(
            w_out[128 * c:128 * (c + 1), :].astype(np.float32)
        )
        in_maps.append(
            {
                "xT": x2.astype(ml_dtypes.bfloat16),
                "w_in_c": w_in_cc.astype(ml_dtypes.bfloat16),
                "w_out_c": w_out_cc.astype(ml_dtypes.bfloat16),
            }
        )

    res = run_bass_kernel_spmd(
        nc, in_maps, core_ids=list(range(NCORES)), trace=_trace
    )
    acc = res.results[0]["po"].astype(np.float64)
    for c in range(1, NCORES):
        acc = acc + res.results[c]["po"].astype(np.float64)
    out = acc.T + b_out.astype(np.float64)
    if _trace:
        kernel.last_result = res
    return np.ascontiguousarray(out.reshape(B, N, DIM).astype(np.float32))
